# revision 7
# baseline (speedup 1.0000x reference)
"""MoE (DeBERTa-style) Trainium2 kernel.

Reference computation (per token):
  scores = x @ gate_w + gate_b            [N, E]
  top2 + softmax over the 2 selected scores
  expert_out_e = gelu(x @ w1[e] + b1[e]) @ w2[e] + b2[e]
  out = sum_e g[:, e] * expert_out_e      (g nonzero only for top-2)

Strategy: data-parallel across 8 NeuronCores (512 tokens each). Each core
computes the gate, the top-2 softmax weights, and the dense per-expert FFN
on-device, combining expert outputs with per-token gate weights.

Layouts (per core, all host-side rearranged for contiguous DMA):
  xt   [H, TOK]          x slice transposed (h on partition for matmuls)
  mm1: psum[f128, tok]   = sum_kh w1[kh,f].T @ xt[kh]          (N=TOK=512)
  gelu+b1 on ACT -> hmid [f128, tok] in SBUF
  mm2: psum[tok128, hw]  = gt.T@b2 (K=6, start) + sum_ft hmid[ft,tok].T @ w2[ft,hw]
  combine: acc[tok, h] (+)= g[:,e] * psum   (DVE scalar_tensor_tensor)

All matmul operands are bitcast to float32r (FP22 truncation, 1 PE pass).
"""

import numpy as np

import concourse.bass as bass
import concourse.bacc as bacc
import concourse.mybir as mybir
import concourse.tile as tile
from concourse.alu_op_type import AluOpType
from concourse.masks import make_identity
from concourse.bass_utils import run_bass_kernel_spmd

F32 = mybir.dt.float32
F32R = mybir.dt.float32r
AF = mybir.ActivationFunctionType

B, S, H, F, E = 8, 512, 768, 3072, 6
NCORES = 8
N = B * S            # 4096 tokens
TOK = N // NCORES    # 512 tokens per core
TT = TOK // 128      # 4 token tiles
KH = H // 128        # 6 h (contraction) tiles
FT = F // 128        # 24 f tiles
HH = 2               # h output split
HHW = H // HH        # 384

NEG_INF = -1.0e30

_CACHE = {}
LAST_RESULTS = None  # test.py reads exec_time_ns off this


def _build_module():
    nc = bacc.Bacc()

    xt_d = nc.dram_tensor("xt", [H, TOK], F32R, kind="ExternalInput")
    gw_d = nc.dram_tensor("gw", [128, KH * E], F32R, kind="ExternalInput")
    gb_d = nc.dram_tensor("gb", [1, E], F32R, kind="ExternalInput")
    w1_d = nc.dram_tensor("w1r", [E, FT, 128, H], F32R, kind="ExternalInput")
    w2_d = nc.dram_tensor("w2r", [E, HH, FT, 128, HHW], F32R, kind="ExternalInput")
    b1_d = nc.dram_tensor("b1r", [128, E * FT], F32, kind="ExternalInput")
    b2_d = nc.dram_tensor("b2r", [E, H], F32R, kind="ExternalInput")
    out_d = nc.dram_tensor("out", [128, TT * H], F32, kind="ExternalOutput")

    with tile.TileContext(nc) as tc:
        with (
            tc.tile_pool(name="const", bufs=1) as cpool,
            tc.tile_pool(name="small", bufs=4) as spool,
            tc.tile_pool(name="hmidp", bufs=30) as hpool,
            tc.tile_pool(name="w1p", bufs=4) as w1pool,
            tc.tile_pool(name="w2p", bufs=4) as w2pool,
            tc.tile_pool(name="psA", bufs=2, space="PSUM") as psA,
            tc.tile_pool(name="psB", bufs=6, space="PSUM") as psB,
        ):
            # ---- constants / inputs resident in SBUF ----
            xt_sb = cpool.tile([128, KH * TOK], F32R, name="xt_sb")
            for kh in range(KH):
                nc.sync.dma_start(
                    out=xt_sb[:, kh * TOK : (kh + 1) * TOK],
                    in_=xt_d[kh * 128 : (kh + 1) * 128, :],
                )
            gw_sb = cpool.tile([128, KH * E], F32R, name="gw_sb")
            nc.sync.dma_start(out=gw_sb, in_=gw_d[:, :])
            gb_sb = cpool.tile([1, E], F32R, name="gb_sb")
            nc.sync.dma_start(out=gb_sb, in_=gb_d[:, :])
            b1_sb = cpool.tile([128, E * FT], F32, name="b1_sb")
            nc.sync.dma_start(out=b1_sb, in_=b1_d[:, :])
            b2_sb = cpool.tile([E, H], F32R, name="b2_sb")
            nc.sync.dma_start(out=b2_sb, in_=b2_d[:, :])

            ones_f32 = cpool.tile([1, 128], F32, name="ones_f32")
            nc.vector.memset(ones_f32, 1.0)
            ones_sb = cpool.tile([1, 128], F32R, name="ones_sb")
            nc.vector.tensor_copy(ones_sb, ones_f32)
            ident = cpool.tile([128, 128], F32, name="ident")
            make_identity(nc, ident)
            g_sb = cpool.tile([128, TT * E], F32, name="g_sb")
            gt_sb = cpool.tile([E, TOK], F32R, name="gt_sb")
            acc_sb = cpool.tile([128, TT * H], F32, name="acc_sb")

            # ---- gate: scores, top-2, softmax weights ----
            for tt in range(TT):
                s_ps = psA.tile([128, E], F32, tag="psA", name="s_ps")
                for kh in range(KH):
                    nc.tensor.matmul(
                        s_ps,
                        xt_sb[:, kh * TOK + tt * 128 : kh * TOK + (tt + 1) * 128],
                        gw_sb[:, kh * E : (kh + 1) * E],
                        start=(kh == 0),
                        stop=False,
                    )
                nc.tensor.matmul(s_ps, ones_sb, gb_sb, start=False, stop=True)

                sa = spool.tile([128, E], F32, name="sa")
                nc.vector.tensor_copy(sa, s_ps)
                m1 = spool.tile([128, 1], F32, name="m1")
                nc.vector.reduce_max(m1, sa, axis=mybir.AxisListType.X)
                eq1 = spool.tile([128, E], F32, name="eq1")
                nc.vector.tensor_scalar(eq1, sa, m1, None, AluOpType.is_equal)
                sm = spool.tile([128, E], F32, name="sm")
                # sm = sa + eq1 * (-1e30): masks out the argmax entry
                nc.vector.scalar_tensor_tensor(
                    sm, eq1, NEG_INF, sa, AluOpType.mult, AluOpType.add
                )
                m2 = spool.tile([128, 1], F32, name="m2")
                nc.vector.reduce_max(m2, sm, axis=mybir.AxisListType.X)
                eq2 = spool.tile([128, E], F32, name="eq2")
                nc.vector.tensor_scalar(eq2, sm, m2, None, AluOpType.is_equal)
                dd = spool.tile([128, 1], F32, name="dd")
                nc.vector.tensor_tensor(dd, m1, m2, AluOpType.subtract)
                wtop = spool.tile([128, 1], F32, name="wtop")
                nc.scalar.activation(wtop, dd, AF.Sigmoid)
                wbot = spool.tile([128, 1], F32, name="wbot")
                nc.vector.tensor_scalar(
                    wbot, wtop, -1.0, 1.0, AluOpType.mult, AluOpType.add
                )
                t1 = spool.tile([128, E], F32, name="t1")
                nc.vector.tensor_scalar(t1, eq1, wtop, None, AluOpType.mult)
                t2 = spool.tile([128, E], F32, name="t2")
                nc.vector.tensor_scalar(t2, eq2, wbot, None, AluOpType.mult)
                nc.vector.tensor_tensor(
                    g_sb[:, tt * E : (tt + 1) * E], t1, t2, AluOpType.add
                )

            # gt[e, tok] = g[tok, e].T via PE transpose
            for tt in range(TT):
                tp = psA.tile([E, 128], F32, tag="psA", name="tp")
                nc.tensor.transpose(tp, g_sb[:, tt * E : (tt + 1) * E], ident)
                nc.vector.tensor_copy(gt_sb[:, tt * 128 : (tt + 1) * 128], tp)

            # ---- dense expert FFN ----
            for e in range(E):
                hm_tiles = []
                for ft in range(FT):
                    w1t = w1pool.tile([128, H], F32R, name="w1t")
                    nc.sync.dma_start(out=w1t, in_=w1_d[e, ft])
                    h_ps = psA.tile([128, TOK], F32, tag="psA", name="h_ps")
                    for kh in range(KH):
                        nc.tensor.matmul(
                            h_ps,
                            w1t[:, kh * 128 : (kh + 1) * 128],
                            xt_sb[:, kh * TOK : (kh + 1) * TOK],
                            start=(kh == 0),
                            stop=(kh == KH - 1),
                        )
                    hm = hpool.tile([128, TOK], F32R, tag="hm", name="hm")
                    nc.scalar.activation(
                        hm, h_ps, AF.Gelu, bias=b1_sb[:, e * FT + ft : e * FT + ft + 1]
                    )
                    hm_tiles.append(hm)

                for hh in range(HH):
                    o_ps_list = []
                    for tt in range(TT):
                        o_ps = psB.tile([128, HHW], F32, tag="psB", name="o_ps")
                        # b2 contribution: gt[:, tok].T @ b2[:, h] (K=E matmul)
                        nc.tensor.matmul(
                            o_ps,
                            gt_sb[:, tt * 128 : (tt + 1) * 128],
                            b2_sb[:, hh * HHW : (hh + 1) * HHW],
                            start=True,
                            stop=False,
                        )
                        o_ps_list.append(o_ps)
                    for ft in range(FT):
                        w2t = w2pool.tile([128, HHW], F32R, name="w2t")
                        nc.sync.dma_start(out=w2t, in_=w2_d[e, hh, ft])
                        for tt in range(TT):
                            nc.tensor.matmul(
                                o_ps_list[tt],
                                hm_tiles[ft][:, tt * 128 : (tt + 1) * 128],
                                w2t,
                                start=False,
                                stop=(ft == FT - 1),
                            )
                    for tt in range(TT):
                        accsl = acc_sb[:, tt * H + hh * HHW : tt * H + (hh + 1) * HHW]
                        gcol = g_sb[:, tt * E + e : tt * E + e + 1]
                        # psum = expert_out_e + sum_e' g_e'*b2_e'; multiplying
                        # by g_e and summing over e yields sum g_e*out_e plus
                        # exactly one combined-b2 term (sum_e g_e == 1).
                        if e == 0:
                            nc.vector.tensor_scalar(
                                accsl, o_ps_list[tt], gcol, None, AluOpType.mult
                            )
                        else:
                            nc.vector.scalar_tensor_tensor(
                                accsl,
                                o_ps_list[tt],
                                gcol,
                                accsl,
                                AluOpType.mult,
                                AluOpType.add,
                            )

            nc.sync.dma_start(out=out_d[:, :], in_=acc_sb[:, :])

    nc.finalize()
    return nc


def _get_module():
    if "nc" not in _CACHE:
        _CACHE["nc"] = _build_module()
    return _CACHE["nc"]


def _prep_shared(gate_w, gate_b, w1, b1, w2, b2):
    """Host-side weight rearrangement into DMA-friendly layouts."""
    gw = np.ascontiguousarray(
        gate_w.reshape(KH, 128, E).transpose(1, 0, 2).reshape(128, KH * E),
        dtype=np.float32,
    )
    gb = np.ascontiguousarray(gate_b.reshape(1, E), dtype=np.float32)
    # w1r[e, ft, p, kh*128+c] = w1[e, kh*128+p, ft*128+c]
    w1r = np.ascontiguousarray(
        w1.reshape(E, KH, 128, FT, 128).transpose(0, 3, 2, 1, 4).reshape(E, FT, 128, H),
        dtype=np.float32,
    )
    # w2r[e, hh, ft, p, c] = w2[e, ft*128+p, hh*HHW+c]
    w2r = np.ascontiguousarray(
        w2.reshape(E, FT, 128, HH, HHW).transpose(0, 3, 1, 2, 4),
        dtype=np.float32,
    )
    # b1r[p, e*FT+ft] = b1[e, ft*128+p]
    b1r = np.ascontiguousarray(
        b1.reshape(E, FT, 128).transpose(2, 0, 1).reshape(128, E * FT),
        dtype=np.float32,
    )
    b2r = np.ascontiguousarray(b2, dtype=np.float32)
    return gw, gb, w1r, w2r, b1r, b2r


def kernel(x, gate_w, gate_b, w1, b1, w2, b2, **run_kwargs):
    global LAST_RESULTS
    x = np.asarray(x, dtype=np.float32)
    gw, gb, w1r, w2r, b1r, b2r = _prep_shared(
        np.asarray(gate_w, np.float32),
        np.asarray(gate_b, np.float32),
        np.asarray(w1, np.float32),
        np.asarray(b1, np.float32),
        np.asarray(w2, np.float32),
        np.asarray(b2, np.float32),
    )
    xf = x.reshape(N, H)
    in_maps = []
    for c in range(NCORES):
        xt = np.ascontiguousarray(xf[c * TOK : (c + 1) * TOK, :].T)
        in_maps.append(
            {
                "xt": xt,
                "gw": gw,
                "gb": gb,
                "w1r": w1r,
                "w2r": w2r,
                "b1r": b1r,
                "b2r": b2r,
            }
        )

    nc = _get_module()
    results = run_bass_kernel_spmd(nc, in_maps, core_ids=list(range(NCORES)), **run_kwargs)
    LAST_RESULTS = results

    out = np.empty((N, H), dtype=np.float32)
    for c in range(NCORES):
        oc = results.results[c]["out"]  # [128, TT*H]
        out[c * TOK : (c + 1) * TOK] = (
            oc.reshape(128, TT, H).transpose(1, 0, 2).reshape(TOK, H)
        )
    return out.reshape(B, S, H)


# revision 9
# speedup vs baseline: 1.2891x; 1.2891x over previous
"""MoE (DeBERTa-style) Trainium2 kernel.

Reference computation (per token):
  scores = x @ gate_w + gate_b            [N, E]
  top2 + softmax over the 2 selected scores
  expert_out_e = gelu(x @ w1[e] + b1[e]) @ w2[e] + b2[e]
  out = sum_e g[:, e] * expert_out_e      (g nonzero only for top-2)

Strategy: data-parallel across 8 NeuronCores (512 tokens each). Each core
computes the gate, the top-2 softmax weights, and the dense per-expert FFN
on-device, combining expert outputs with per-token gate weights.

Layouts (per core, all host-side rearranged for contiguous DMA):
  xt   [H, TOK]          x slice transposed (h on partition for matmuls)
  mm1: psum[f128, tok]   = sum_kh w1[kh,f].T @ xt[kh]          (N=TOK=512)
  gelu+b1 on ACT -> hmid [f128, tok] in SBUF
  mm2: psum[tok128, hw]  = gt.T@b2 (K=6, start) + sum_ft hmid[ft,tok].T @ w2[ft,hw]
  combine: acc[tok, h] (+)= g[:,e] * psum   (DVE scalar_tensor_tensor)

All matmul operands are bitcast to float32r (FP22 truncation, 1 PE pass).
"""

import numpy as np
import ml_dtypes

import concourse.bass as bass
import concourse.bacc as bacc
import concourse.mybir as mybir
import concourse.tile as tile
from concourse.alu_op_type import AluOpType
from concourse.masks import make_identity
from concourse.bass_utils import run_bass_kernel_spmd

F32 = mybir.dt.float32
F32R = mybir.dt.float32r
BF16 = mybir.dt.bfloat16
AF = mybir.ActivationFunctionType

B, S, H, F, E = 8, 512, 768, 3072, 6
NCORES = 8
N = B * S            # 4096 tokens
TOK = N // NCORES    # 512 tokens per core
TT = TOK // 128      # 4 token tiles
KH = H // 128        # 6 h (contraction) tiles
FT = F // 128        # 24 f tiles
HH = 2               # h output split
HHW = H // HH        # 384

NEG_INF = -1.0e30

_CACHE = {}
LAST_RESULTS = None  # test.py reads exec_time_ns off this


def _build_module():
    nc = bacc.Bacc()

    xt_d = nc.dram_tensor("xt", [H, TOK], F32R, kind="ExternalInput")
    xtb_d = nc.dram_tensor("xtb", [H, TOK], BF16, kind="ExternalInput")
    gw_d = nc.dram_tensor("gw", [128, KH * E], F32R, kind="ExternalInput")
    gb_d = nc.dram_tensor("gb", [1, E], F32R, kind="ExternalInput")
    w1_d = nc.dram_tensor("w1r", [E, FT, 128, H], BF16, kind="ExternalInput")
    w2_d = nc.dram_tensor("w2r", [E, HH, FT, 128, HHW], BF16, kind="ExternalInput")
    b1_d = nc.dram_tensor("b1r", [128, E * FT], F32, kind="ExternalInput")
    b2_d = nc.dram_tensor("b2r", [E, H], BF16, kind="ExternalInput")
    out_d = nc.dram_tensor("out", [128, TT * H], F32, kind="ExternalOutput")

    with tile.TileContext(nc) as tc:
        with (
            tc.tile_pool(name="const", bufs=1) as cpool,
            tc.tile_pool(name="small", bufs=4) as spool,
            tc.tile_pool(name="hmidp", bufs=30) as hpool,
            tc.tile_pool(name="w1p", bufs=4) as w1pool,
            tc.tile_pool(name="w2p", bufs=4) as w2pool,
            tc.tile_pool(name="psA", bufs=2, space="PSUM") as psA,
            tc.tile_pool(name="psB", bufs=6, space="PSUM") as psB,
        ):
            # ---- constants / inputs resident in SBUF ----
            xt_sb = cpool.tile([128, KH * TOK], F32R, name="xt_sb")
            for kh in range(KH):
                nc.sync.dma_start(
                    out=xt_sb[:, kh * TOK : (kh + 1) * TOK],
                    in_=xt_d[kh * 128 : (kh + 1) * 128, :],
                )
            xtb_sb = cpool.tile([128, KH * TOK], BF16, name="xtb_sb")
            for kh in range(KH):
                nc.sync.dma_start(
                    out=xtb_sb[:, kh * TOK : (kh + 1) * TOK],
                    in_=xtb_d[kh * 128 : (kh + 1) * 128, :],
                )
            gw_sb = cpool.tile([128, KH * E], F32R, name="gw_sb")
            nc.sync.dma_start(out=gw_sb, in_=gw_d[:, :])
            gb_sb = cpool.tile([1, E], F32R, name="gb_sb")
            nc.sync.dma_start(out=gb_sb, in_=gb_d[:, :])
            b1_sb = cpool.tile([128, E * FT], F32, name="b1_sb")
            nc.sync.dma_start(out=b1_sb, in_=b1_d[:, :])
            b2_sb = cpool.tile([E, H], BF16, name="b2_sb")
            nc.sync.dma_start(out=b2_sb, in_=b2_d[:, :])

            ones_f32 = cpool.tile([1, 128], F32, name="ones_f32")
            nc.vector.memset(ones_f32, 1.0)
            ones_sb = cpool.tile([1, 128], F32R, name="ones_sb")
            nc.vector.tensor_copy(ones_sb, ones_f32)
            ident = cpool.tile([128, 128], F32, name="ident")
            make_identity(nc, ident)
            g_sb = cpool.tile([128, TT * E], F32, name="g_sb")
            gt_sb = cpool.tile([E, TOK], BF16, name="gt_sb")
            acc_sb = cpool.tile([128, TT * H], F32, name="acc_sb")

            # ---- gate: scores, top-2, softmax weights ----
            for tt in range(TT):
                s_ps = psA.tile([128, E], F32, tag="psA", name="s_ps")
                for kh in range(KH):
                    nc.tensor.matmul(
                        s_ps,
                        xt_sb[:, kh * TOK + tt * 128 : kh * TOK + (tt + 1) * 128],
                        gw_sb[:, kh * E : (kh + 1) * E],
                        start=(kh == 0),
                        stop=False,
                    )
                nc.tensor.matmul(s_ps, ones_sb, gb_sb, start=False, stop=True)

                sa = spool.tile([128, E], F32, name="sa")
                nc.vector.tensor_copy(sa, s_ps)
                m1 = spool.tile([128, 1], F32, name="m1")
                nc.vector.reduce_max(m1, sa, axis=mybir.AxisListType.X)
                eq1 = spool.tile([128, E], F32, name="eq1")
                nc.vector.tensor_scalar(eq1, sa, m1, None, AluOpType.is_equal)
                sm = spool.tile([128, E], F32, name="sm")
                # sm = sa + eq1 * (-1e30): masks out the argmax entry
                nc.vector.scalar_tensor_tensor(
                    sm, eq1, NEG_INF, sa, AluOpType.mult, AluOpType.add
                )
                m2 = spool.tile([128, 1], F32, name="m2")
                nc.vector.reduce_max(m2, sm, axis=mybir.AxisListType.X)
                eq2 = spool.tile([128, E], F32, name="eq2")
                nc.vector.tensor_scalar(eq2, sm, m2, None, AluOpType.is_equal)
                dd = spool.tile([128, 1], F32, name="dd")
                nc.vector.tensor_tensor(dd, m1, m2, AluOpType.subtract)
                wtop = spool.tile([128, 1], F32, name="wtop")
                nc.scalar.activation(wtop, dd, AF.Sigmoid)
                wbot = spool.tile([128, 1], F32, name="wbot")
                nc.vector.tensor_scalar(
                    wbot, wtop, -1.0, 1.0, AluOpType.mult, AluOpType.add
                )
                t1 = spool.tile([128, E], F32, name="t1")
                nc.vector.tensor_scalar(t1, eq1, wtop, None, AluOpType.mult)
                t2 = spool.tile([128, E], F32, name="t2")
                nc.vector.tensor_scalar(t2, eq2, wbot, None, AluOpType.mult)
                nc.vector.tensor_tensor(
                    g_sb[:, tt * E : (tt + 1) * E], t1, t2, AluOpType.add
                )

            # gt[e, tok] = g[tok, e].T via PE transpose
            for tt in range(TT):
                tp = psA.tile([E, 128], F32, tag="psA", name="tp")
                nc.tensor.transpose(tp, g_sb[:, tt * E : (tt + 1) * E], ident)
                nc.vector.tensor_copy(gt_sb[:, tt * 128 : (tt + 1) * 128], tp)

            # ---- dense expert FFN ----
            for e in range(E):
                hm_tiles = []
                for ft in range(FT):
                    w1t = w1pool.tile([128, H], BF16, name="w1t")
                    nc.sync.dma_start(out=w1t, in_=w1_d[e, ft])
                    h_ps = psA.tile([128, TOK], F32, tag="psA", name="h_ps")
                    for kh in range(KH):
                        nc.tensor.matmul(
                            h_ps,
                            w1t[:, kh * 128 : (kh + 1) * 128],
                            xtb_sb[:, kh * TOK : (kh + 1) * TOK],
                            start=(kh == 0),
                            stop=(kh == KH - 1),
                        )
                    hm = hpool.tile([128, TOK], BF16, tag="hm", name="hm")
                    nc.scalar.activation(
                        hm, h_ps, AF.Gelu, bias=b1_sb[:, e * FT + ft : e * FT + ft + 1]
                    )
                    hm_tiles.append(hm)

                for hh in range(HH):
                    o_ps_list = []
                    for tt in range(TT):
                        o_ps = psB.tile([128, HHW], F32, tag="psB", name="o_ps")
                        # b2 contribution: gt[:, tok].T @ b2[:, h] (K=E matmul)
                        nc.tensor.matmul(
                            o_ps,
                            gt_sb[:, tt * 128 : (tt + 1) * 128],
                            b2_sb[:, hh * HHW : (hh + 1) * HHW],
                            start=True,
                            stop=False,
                        )
                        o_ps_list.append(o_ps)
                    for ft in range(FT):
                        w2t = w2pool.tile([128, HHW], BF16, name="w2t")
                        nc.sync.dma_start(out=w2t, in_=w2_d[e, hh, ft])
                        for tt in range(TT):
                            nc.tensor.matmul(
                                o_ps_list[tt],
                                hm_tiles[ft][:, tt * 128 : (tt + 1) * 128],
                                w2t,
                                start=False,
                                stop=(ft == FT - 1),
                            )
                    for tt in range(TT):
                        accsl = acc_sb[:, tt * H + hh * HHW : tt * H + (hh + 1) * HHW]
                        gcol = g_sb[:, tt * E + e : tt * E + e + 1]
                        # psum = expert_out_e + sum_e' g_e'*b2_e'; multiplying
                        # by g_e and summing over e yields sum g_e*out_e plus
                        # exactly one combined-b2 term (sum_e g_e == 1).
                        if e == 0:
                            nc.vector.tensor_scalar(
                                accsl, o_ps_list[tt], gcol, None, AluOpType.mult
                            )
                        else:
                            nc.vector.scalar_tensor_tensor(
                                accsl,
                                o_ps_list[tt],
                                gcol,
                                accsl,
                                AluOpType.mult,
                                AluOpType.add,
                            )

            nc.sync.dma_start(out=out_d[:, :], in_=acc_sb[:, :])

    nc.finalize()
    return nc


def _get_module():
    if "nc" not in _CACHE:
        _CACHE["nc"] = _build_module()
    return _CACHE["nc"]


def _prep_shared(gate_w, gate_b, w1, b1, w2, b2):
    """Host-side weight rearrangement into DMA-friendly layouts."""
    gw = np.ascontiguousarray(
        gate_w.reshape(KH, 128, E).transpose(1, 0, 2).reshape(128, KH * E),
        dtype=np.float32,
    )
    gb = np.ascontiguousarray(gate_b.reshape(1, E), dtype=np.float32)
    # w1r[e, ft, p, kh*128+c] = w1[e, kh*128+p, ft*128+c]
    w1r = np.ascontiguousarray(
        w1.reshape(E, KH, 128, FT, 128).transpose(0, 3, 2, 1, 4).reshape(E, FT, 128, H),
        dtype=ml_dtypes.bfloat16,
    )
    # w2r[e, hh, ft, p, c] = w2[e, ft*128+p, hh*HHW+c]
    w2r = np.ascontiguousarray(
        w2.reshape(E, FT, 128, HH, HHW).transpose(0, 3, 1, 2, 4),
        dtype=ml_dtypes.bfloat16,
    )
    # b1r[p, e*FT+ft] = b1[e, ft*128+p]
    b1r = np.ascontiguousarray(
        b1.reshape(E, FT, 128).transpose(2, 0, 1).reshape(128, E * FT),
        dtype=np.float32,
    )
    b2r = np.ascontiguousarray(b2, dtype=ml_dtypes.bfloat16)
    return gw, gb, w1r, w2r, b1r, b2r


def kernel(x, gate_w, gate_b, w1, b1, w2, b2, **run_kwargs):
    global LAST_RESULTS
    x = np.asarray(x, dtype=np.float32)
    gw, gb, w1r, w2r, b1r, b2r = _prep_shared(
        np.asarray(gate_w, np.float32),
        np.asarray(gate_b, np.float32),
        np.asarray(w1, np.float32),
        np.asarray(b1, np.float32),
        np.asarray(w2, np.float32),
        np.asarray(b2, np.float32),
    )
    xf = x.reshape(N, H)
    in_maps = []
    for c in range(NCORES):
        xt = np.ascontiguousarray(xf[c * TOK : (c + 1) * TOK, :].T)
        xtb = np.ascontiguousarray(xt, dtype=ml_dtypes.bfloat16)
        in_maps.append(
            {
                "xt": xt,
                "xtb": xtb,
                "gw": gw,
                "gb": gb,
                "w1r": w1r,
                "w2r": w2r,
                "b1r": b1r,
                "b2r": b2r,
            }
        )

    nc = _get_module()
    results = run_bass_kernel_spmd(nc, in_maps, core_ids=list(range(NCORES)), **run_kwargs)
    LAST_RESULTS = results

    out = np.empty((N, H), dtype=np.float32)
    for c in range(NCORES):
        oc = results.results[c]["out"]  # [128, TT*H]
        out[c * TOK : (c + 1) * TOK] = (
            oc.reshape(128, TT, H).transpose(1, 0, 2).reshape(TOK, H)
        )
    return out.reshape(B, S, H)


# revision 15
# speedup vs baseline: 1.2974x; 1.0064x over previous
"""MoE (DeBERTa-style) Trainium2 kernel.

Reference computation (per token):
  scores = x @ gate_w + gate_b            [N, E]
  top2 + softmax over the 2 selected scores
  expert_out_e = gelu(x @ w1[e] + b1[e]) @ w2[e] + b2[e]
  out = sum_e g[:, e] * expert_out_e      (g nonzero only for top-2)

Strategy: data-parallel across 8 NeuronCores (512 tokens each). Each core
computes the gate, the top-2 softmax weights, and the dense per-expert FFN
on-device, combining expert outputs with per-token gate weights.

Layouts (per core, all host-side rearranged for contiguous DMA):
  xt   [H, TOK]          x slice transposed (h on partition for matmuls)
  mm1: psum[f128, tok]   = sum_kh w1[kh,f].T @ xt[kh]          (N=TOK=512)
  gelu+b1 on ACT -> hmid [f128, tok] in SBUF
  mm2: psum[tok128, hw]  = gt.T@b2 (K=6, start) + sum_ft hmid[ft,tok].T @ w2[ft,hw]
  combine: acc[tok, h] (+)= g[:,e] * psum   (DVE scalar_tensor_tensor)

All matmul operands are bitcast to float32r (FP22 truncation, 1 PE pass).
"""

import os

import numpy as np
import ml_dtypes

import concourse.bass as bass
import concourse.bacc as bacc
import concourse.mybir as mybir
import concourse.tile as tile
from concourse.alu_op_type import AluOpType
from concourse.masks import make_identity
from concourse.bass import IndirectOffsetOnAxis
from concourse.bass_utils import run_bass_kernel_spmd

F32 = mybir.dt.float32
F32R = mybir.dt.float32r
BF16 = mybir.dt.bfloat16
AF = mybir.ActivationFunctionType

B, S, H, F, E = 8, 512, 768, 3072, 6
NCORES = 8
N = B * S            # 4096 tokens
TOK = N // NCORES    # 512 tokens per core
TT = TOK // 128      # 4 token tiles
KH = H // 128        # 6 h (contraction) tiles
FT = F // 128        # 24 f tiles
HH = 2               # h output split
HHW = H // HH        # 384

NEG_INF = -1.0e30
I32 = mybir.dt.int32
C = 256              # routed: bucket capacity per expert
ST = C // 128
TRASH = 2 * TOK
MODE = os.environ.get("MOE_MODE", "dense")

_CACHE = {}
LAST_RESULTS = None  # test.py reads exec_time_ns off this


def _build_dense():
    nc = bacc.Bacc()

    xt_d = nc.dram_tensor("xt", [H, TOK], F32R, kind="ExternalInput")
    xtb_d = nc.dram_tensor("xtb", [H, TOK], BF16, kind="ExternalInput")
    gw_d = nc.dram_tensor("gw", [128, KH * E], F32R, kind="ExternalInput")
    gb_d = nc.dram_tensor("gb", [1, E], F32R, kind="ExternalInput")
    w1_d = nc.dram_tensor("w1r", [E, FT, 128, H], BF16, kind="ExternalInput")
    w2_d = nc.dram_tensor("w2r", [E, HH, FT, 128, HHW], BF16, kind="ExternalInput")
    b1_d = nc.dram_tensor("b1r", [128, E * FT], F32, kind="ExternalInput")
    b2_d = nc.dram_tensor("b2r", [E, H], BF16, kind="ExternalInput")
    out_d = nc.dram_tensor("out", [128, TT * H], F32, kind="ExternalOutput")

    with tile.TileContext(nc) as tc:
        with (
            tc.tile_pool(name="const", bufs=1) as cpool,
            tc.tile_pool(name="small", bufs=4) as spool,
            tc.tile_pool(name="hmidp", bufs=30) as hpool,
            tc.tile_pool(name="w1p", bufs=4) as w1pool,
            tc.tile_pool(name="w2p", bufs=4) as w2pool,
            tc.tile_pool(name="psA", bufs=2, space="PSUM") as psA,
            tc.tile_pool(name="psB", bufs=6, space="PSUM") as psB,
        ):
            # ---- constants / inputs resident in SBUF ----
            xt_sb = cpool.tile([128, KH * TOK], F32R, name="xt_sb")
            for kh in range(KH):
                nc.sync.dma_start(
                    out=xt_sb[:, kh * TOK : (kh + 1) * TOK],
                    in_=xt_d[kh * 128 : (kh + 1) * 128, :],
                )
            xtb_sb = cpool.tile([128, KH * TOK], BF16, name="xtb_sb")
            for kh in range(KH):
                nc.sync.dma_start(
                    out=xtb_sb[:, kh * TOK : (kh + 1) * TOK],
                    in_=xtb_d[kh * 128 : (kh + 1) * 128, :],
                )
            gw_sb = cpool.tile([128, KH * E], F32R, name="gw_sb")
            nc.sync.dma_start(out=gw_sb, in_=gw_d[:, :])
            gb_sb = cpool.tile([1, E], F32R, name="gb_sb")
            nc.sync.dma_start(out=gb_sb, in_=gb_d[:, :])
            b1_sb = cpool.tile([128, E * FT], F32, name="b1_sb")
            nc.sync.dma_start(out=b1_sb, in_=b1_d[:, :])
            b2_sb = cpool.tile([E, H], BF16, name="b2_sb")
            nc.sync.dma_start(out=b2_sb, in_=b2_d[:, :])

            ones_f32 = cpool.tile([1, 128], F32, name="ones_f32")
            nc.vector.memset(ones_f32, 1.0)
            ones_sb = cpool.tile([1, 128], F32R, name="ones_sb")
            nc.vector.tensor_copy(ones_sb, ones_f32)
            ident = cpool.tile([128, 128], F32, name="ident")
            make_identity(nc, ident)
            g_sb = cpool.tile([128, TT * E], F32, name="g_sb")
            gt_sb = cpool.tile([E, TOK], BF16, name="gt_sb")
            acc_sb = cpool.tile([128, TT * H], F32, name="acc_sb")

            # ---- gate: scores, top-2, softmax weights ----
            for tt in range(TT):
                s_ps = psA.tile([128, E], F32, tag="psA", name="s_ps")
                for kh in range(KH):
                    nc.tensor.matmul(
                        s_ps,
                        xt_sb[:, kh * TOK + tt * 128 : kh * TOK + (tt + 1) * 128],
                        gw_sb[:, kh * E : (kh + 1) * E],
                        start=(kh == 0),
                        stop=False,
                    )
                nc.tensor.matmul(s_ps, ones_sb, gb_sb, start=False, stop=True)

                sa = spool.tile([128, E], F32, name="sa")
                nc.vector.tensor_copy(sa, s_ps)
                m1 = spool.tile([128, 1], F32, name="m1")
                nc.vector.reduce_max(m1, sa, axis=mybir.AxisListType.X)
                eq1 = spool.tile([128, E], F32, name="eq1")
                nc.vector.tensor_scalar(eq1, sa, m1, None, AluOpType.is_equal)
                sm = spool.tile([128, E], F32, name="sm")
                # sm = sa + eq1 * (-1e30): masks out the argmax entry
                nc.vector.scalar_tensor_tensor(
                    sm, eq1, NEG_INF, sa, AluOpType.mult, AluOpType.add
                )
                m2 = spool.tile([128, 1], F32, name="m2")
                nc.vector.reduce_max(m2, sm, axis=mybir.AxisListType.X)
                eq2 = spool.tile([128, E], F32, name="eq2")
                nc.vector.tensor_scalar(eq2, sm, m2, None, AluOpType.is_equal)
                dd = spool.tile([128, 1], F32, name="dd")
                nc.vector.tensor_tensor(dd, m1, m2, AluOpType.subtract)
                wtop = spool.tile([128, 1], F32, name="wtop")
                nc.scalar.activation(wtop, dd, AF.Sigmoid)
                wbot = spool.tile([128, 1], F32, name="wbot")
                nc.vector.tensor_scalar(
                    wbot, wtop, -1.0, 1.0, AluOpType.mult, AluOpType.add
                )
                t1 = spool.tile([128, E], F32, name="t1")
                nc.vector.tensor_scalar(t1, eq1, wtop, None, AluOpType.mult)
                t2 = spool.tile([128, E], F32, name="t2")
                nc.vector.tensor_scalar(t2, eq2, wbot, None, AluOpType.mult)
                nc.vector.tensor_tensor(
                    g_sb[:, tt * E : (tt + 1) * E], t1, t2, AluOpType.add
                )

            # ---- dense expert FFN ----
            for e in range(E):
                hm_tiles = []
                for ft in range(FT):
                    w1t = w1pool.tile([128, H], BF16, name="w1t")
                    nc.sync.dma_start(out=w1t, in_=w1_d[e, ft])
                    h_ps = psA.tile([128, TOK], F32, tag="psA", name="h_ps")
                    for kh in range(KH):
                        nc.tensor.matmul(
                            h_ps,
                            w1t[:, kh * 128 : (kh + 1) * 128],
                            xtb_sb[:, kh * TOK : (kh + 1) * TOK],
                            start=(kh == 0),
                            stop=(kh == KH - 1),
                        )
                    hm = hpool.tile([128, TOK], BF16, tag="hm", name="hm")
                    nc.scalar.activation(
                        hm, h_ps, AF.Gelu, bias=b1_sb[:, e * FT + ft : e * FT + ft + 1]
                    )
                    hm_tiles.append(hm)

                if e == 0:
                    # gt[e, tok] = g[tok, e].T via PE transpose; emitted after
                    # expert 0's mm1 stream so PE doesn't stall on the DVE
                    # gate chain at kernel start (PE queue is FIFO).
                    for tt in range(TT):
                        tp = psA.tile([E, 128], F32, tag="psA", name="tp")
                        nc.tensor.transpose(
                            tp, g_sb[:, tt * E : (tt + 1) * E], ident
                        )
                        nc.vector.tensor_copy(
                            gt_sb[:, tt * 128 : (tt + 1) * 128], tp
                        )

                for hh in range(HH):
                    o_ps_list = []
                    for tt in range(TT):
                        o_ps = psB.tile([128, HHW], F32, tag="psB", name="o_ps")
                        # b2 contribution: gt[:, tok].T @ b2[:, h] (K=E matmul)
                        nc.tensor.matmul(
                            o_ps,
                            gt_sb[:, tt * 128 : (tt + 1) * 128],
                            b2_sb[:, hh * HHW : (hh + 1) * HHW],
                            start=True,
                            stop=False,
                        )
                        o_ps_list.append(o_ps)
                    for ft in range(FT):
                        w2t = w2pool.tile([128, HHW], BF16, name="w2t")
                        nc.sync.dma_start(out=w2t, in_=w2_d[e, hh, ft])
                        for tt in range(TT):
                            nc.tensor.matmul(
                                o_ps_list[tt],
                                hm_tiles[ft][:, tt * 128 : (tt + 1) * 128],
                                w2t,
                                start=False,
                                stop=(ft == FT - 1),
                            )
                    for tt in range(TT):
                        accsl = acc_sb[:, tt * H + hh * HHW : tt * H + (hh + 1) * HHW]
                        gcol = g_sb[:, tt * E + e : tt * E + e + 1]
                        # psum = expert_out_e + sum_e' g_e'*b2_e'; multiplying
                        # by g_e and summing over e yields sum g_e*out_e plus
                        # exactly one combined-b2 term (sum_e g_e == 1).
                        if e == 0:
                            nc.vector.tensor_scalar(
                                accsl, o_ps_list[tt], gcol, None, AluOpType.mult
                            )
                        else:
                            nc.vector.scalar_tensor_tensor(
                                accsl,
                                o_ps_list[tt],
                                gcol,
                                accsl,
                                AluOpType.mult,
                                AluOpType.add,
                            )

            for tt in range(TT):
                nc.sync.dma_start(
                    out=out_d[:, tt * H : (tt + 1) * H],
                    in_=acc_sb[:, tt * H : (tt + 1) * H],
                )

    nc.finalize()
    return nc


def _build_routed():
    nc = bacc.Bacc()

    xt_d = nc.dram_tensor("xt", [H, TOK], F32R, kind="ExternalInput")
    xr_d = nc.dram_tensor("xr", [TOK + 1, H], BF16, kind="ExternalInput")
    gw_d = nc.dram_tensor("gw", [128, KH * E], F32R, kind="ExternalInput")
    gb_d = nc.dram_tensor("gb", [1, E], F32R, kind="ExternalInput")
    w1_d = nc.dram_tensor("w1r", [E, FT, 128, H], BF16, kind="ExternalInput")
    w2_d = nc.dram_tensor("w2r", [E, HH, FT, 128, HHW], BF16, kind="ExternalInput")
    b1_d = nc.dram_tensor("b1r", [128, E * FT], F32, kind="ExternalInput")
    b2_d = nc.dram_tensor("b2rr", [1, E * H], BF16, kind="ExternalInput")
    iota_d = nc.dram_tensor("iota", [128, TT], F32, kind="ExternalInput")
    trash_d = nc.dram_tensor("trash", [TOK + 1, 1], I32, kind="ExternalInput")
    out_d = nc.dram_tensor("out", [128, TT * H], F32, kind="ExternalOutput")

    with tile.TileContext(nc) as tc:
        with (
            tc.tile_pool(name="const", bufs=1) as cpool,
            tc.tile_pool(name="small", bufs=4) as spool,
            tc.tile_pool(name="idxp", bufs=8) as ipool,
            tc.tile_pool(name="xgp", bufs=3) as xgpool,
            tc.tile_pool(name="xtgp", bufs=6) as xtgpool,
            tc.tile_pool(name="hmgp", bufs=28) as hpool,
            tc.tile_pool(name="w1p", bufs=4) as w1pool,
            tc.tile_pool(name="w2p", bufs=4) as w2pool,
            tc.tile_pool(name="o2p", bufs=3) as o2pool,
            tc.tile_pool(name="psA", bufs=2, space="PSUM") as psA,
            tc.tile_pool(name="psB", bufs=6, space="PSUM") as psB,
            tc.tile_pool(name="dramp", bufs=1, space="DRAM") as dpool,
        ):
            # ---- DRAM staging ----
            y2_dr = dpool.tile([2 * TOK + 1, H], F32, name="y2_dr")
            offb_dr = [
                dpool.tile([TOK + 1, 1], I32, name=f"offb{e}_dr") for e in range(E)
            ]

            # ---- constants / resident inputs ----
            xt_sb = cpool.tile([128, KH * TOK], F32R, name="xt_sb")
            for kh in range(KH):
                nc.sync.dma_start(
                    out=xt_sb[:, kh * TOK : (kh + 1) * TOK],
                    in_=xt_d[kh * 128 : (kh + 1) * 128, :],
                )
            gw_sb = cpool.tile([128, KH * E], F32R, name="gw_sb")
            nc.sync.dma_start(out=gw_sb, in_=gw_d[:, :])
            gb_sb = cpool.tile([1, E], F32R, name="gb_sb")
            nc.sync.dma_start(out=gb_sb, in_=gb_d[:, :])
            b1_sb = cpool.tile([128, E * FT], F32, name="b1_sb")
            nc.sync.dma_start(out=b1_sb, in_=b1_d[:, :])
            b2_sb = cpool.tile([1, E * H], BF16, name="b2_sb")
            nc.sync.dma_start(out=b2_sb, in_=b2_d[:, :])
            iota_sb = cpool.tile([128, TT], F32, name="iota_sb")
            nc.sync.dma_start(out=iota_sb, in_=iota_d[:, :])

            ones_f32 = cpool.tile([1, 128], F32, name="ones_f32")
            nc.vector.memset(ones_f32, 1.0)
            ones_r = cpool.tile([1, 128], F32R, name="ones_r")
            nc.vector.tensor_copy(ones_r, ones_f32)
            ones_bf = cpool.tile([1, 128], BF16, name="ones_bf")
            nc.vector.tensor_copy(ones_bf, ones_f32)
            ident = cpool.tile([128, 128], F32, name="ident")
            make_identity(nc, ident)

            wt_sb = cpool.tile([128, TT], F32, name="wt_sb")
            wb_sb = cpool.tile([128, TT], F32, name="wb_sb")
            mask_sb = cpool.tile([128, TT * E], F32, name="mask_sb")
            eq2_sb = cpool.tile([128, TT * E], F32, name="eq2_sb")
            maskT_sb = cpool.tile([E, TOK], F32, name="maskT_sb")
            scanT_sb = cpool.tile([E, TOK], F32, name="scanT_sb")
            valf_sb = cpool.tile([128, TT * E], F32, name="valf_sb")
            offf_sb = cpool.tile([128, TT * E], F32, name="offf_sb")
            vali_sb = cpool.tile([128, TT * E], I32, name="vali_sb")
            offi_sb = cpool.tile([128, TT * E], I32, name="offi_sb")
            acc_sb = cpool.tile([128, TT * H], F32, name="acc_sb")
            bufA_sb = cpool.tile([128, TT * H], F32, name="bufA_sb")
            bufB_sb = cpool.tile([128, TT * H], F32, name="bufB_sb")

            # prefill offset buffers with the pad marker
            for e in range(E):
                nc.sync.dma_start(out=offb_dr[e][:, :], in_=trash_d[:, :])

            # ---- gate: scores, top-2, masks, softmax weights ----
            for tt in range(TT):
                s_ps = psA.tile([128, E], F32, tag="psA", name="s_ps")
                for kh in range(KH):
                    nc.tensor.matmul(
                        s_ps,
                        xt_sb[:, kh * TOK + tt * 128 : kh * TOK + (tt + 1) * 128],
                        gw_sb[:, kh * E : (kh + 1) * E],
                        start=(kh == 0),
                        stop=False,
                    )
                nc.tensor.matmul(s_ps, ones_r, gb_sb, start=False, stop=True)

                sa = spool.tile([128, E], F32, name="sa")
                nc.vector.tensor_copy(sa, s_ps)
                m1 = spool.tile([128, 1], F32, name="m1")
                nc.vector.reduce_max(m1, sa, axis=mybir.AxisListType.X)
                eq1 = spool.tile([128, E], F32, name="eq1")
                nc.vector.tensor_scalar(eq1, sa, m1, None, AluOpType.is_equal)
                sm = spool.tile([128, E], F32, name="sm")
                nc.vector.scalar_tensor_tensor(
                    sm, eq1, NEG_INF, sa, AluOpType.mult, AluOpType.add
                )
                m2 = spool.tile([128, 1], F32, name="m2")
                nc.vector.reduce_max(m2, sm, axis=mybir.AxisListType.X)
                eq2sl = eq2_sb[:, tt * E : (tt + 1) * E]
                nc.vector.tensor_scalar(eq2sl, sm, m2, None, AluOpType.is_equal)
                dd = spool.tile([128, 1], F32, name="dd")
                nc.vector.tensor_tensor(dd, m1, m2, AluOpType.subtract)
                nc.scalar.activation(wt_sb[:, tt : tt + 1], dd, AF.Sigmoid)
                nc.vector.tensor_scalar(
                    wb_sb[:, tt : tt + 1],
                    wt_sb[:, tt : tt + 1],
                    -1.0,
                    1.0,
                    AluOpType.mult,
                    AluOpType.add,
                )
                # mask = eq1 + eq2; val = iota + TOK*eq2
                masksl = mask_sb[:, tt * E : (tt + 1) * E]
                nc.vector.tensor_tensor(masksl, eq1, eq2sl, AluOpType.add)
                nc.vector.tensor_scalar(
                    valf_sb.rearrange("p (e t) -> p e t", t=TT)[:, :, tt],
                    eq2sl,
                    float(TOK),
                    iota_sb[:, tt : tt + 1],
                    AluOpType.mult,
                    AluOpType.add,
                )

            # ---- bucket positions: transpose -> scan -> transpose back ----
            for tt in range(TT):
                mt_ps = psA.tile([E, 128], F32, tag="psA", name="mt_ps")
                nc.tensor.transpose(
                    mt_ps, mask_sb[:, tt * E : (tt + 1) * E], ident
                )
                nc.vector.tensor_copy(
                    maskT_sb[:, tt * 128 : (tt + 1) * 128], mt_ps
                )
            # inclusive prefix sum along tokens (free dim), per expert row
            nc.vector.tensor_tensor_scan(
                scanT_sb[:, :],
                maskT_sb[:, :],
                maskT_sb[:, :],
                0.0,
                AluOpType.add,
                AluOpType.bypass,
            )
            for tt in range(TT):
                pt_ps = psA.tile([128, E], F32, tag="psA", name="pt_ps")
                # transpose back a [E, 128] strip -> [128, E]
                nc.tensor.transpose(
                    pt_ps, scanT_sb[:, tt * 128 : (tt + 1) * 128], ident[0:E, 0:E]
                )
                posin = spool.tile([128, E], F32, name="posin")
                nc.vector.tensor_copy(posin, pt_ps)
                masksl = mask_sb[:, tt * E : (tt + 1) * E]
                # exclusive position = inclusive - mask
                posx = spool.tile([128, E], F32, name="posx")
                nc.vector.tensor_tensor(posx, posin, masksl, AluOpType.subtract)
                # offsc = mask * (posx - C) + C  (pad/unselected -> C = trash)
                u = spool.tile([128, E], F32, name="u")
                nc.vector.tensor_scalar(u, posx, float(-C), None, AluOpType.add)
                v = spool.tile([128, E], F32, name="v")
                nc.vector.tensor_tensor(v, u, masksl, AluOpType.mult)
                nc.vector.tensor_scalar(
                    offf_sb.rearrange("p (e t) -> p e t", t=TT)[:, :, tt],
                    v,
                    float(C),
                    None,
                    AluOpType.add,
                )
            nc.vector.tensor_copy(vali_sb[:, :], valf_sb[:, :])
            nc.vector.tensor_copy(offi_sb[:, :], offf_sb[:, :])

            # ---- scatter slot->tokenvalue tables, one bucket per expert ----
            for e in range(E):
                nc.gpsimd.indirect_dma_start(
                    out=offb_dr[e][:, :],
                    out_offset=IndirectOffsetOnAxis(
                        ap=offi_sb[:, e * TT : (e + 1) * TT], axis=0
                    ),
                    in_=vali_sb[:, e * TT : (e + 1) * TT],
                    in_offset=None,
                )

            # ---- routing prep for ALL experts upfront (keeps gpsimd queue
            # ahead of the FFN so no expert waits on a gather) ----
            evals = []
            extg = []
            for e in range(E):
                vld = ipool.tile([128, ST], I32, name="vld")
                nc.sync.dma_start(
                    out=vld,
                    in_=offb_dr[e][0:C, :].rearrange("(s p) o -> p (s o)", p=128),
                )
                evals.append(vld)
                vf = ipool.tile([128, ST], F32, name="vf")
                nc.vector.tensor_copy(vf, vld)
                flag = ipool.tile([128, ST], F32, name="flag")
                nc.vector.tensor_scalar(flag, vf, float(TOK), None, AluOpType.is_ge)
                tokf = ipool.tile([128, ST], F32, name="tokf")
                nc.vector.scalar_tensor_tensor(
                    tokf, flag, float(-TOK), vf, AluOpType.mult, AluOpType.add
                )
                toki = ipool.tile([128, ST], I32, name="toki")
                nc.vector.tensor_copy(toki, tokf)
                xg = xgpool.tile([128, ST * H], BF16, name="xg")
                nc.gpsimd.indirect_dma_start(
                    out=xg.rearrange("p (s h) -> p s h", s=ST),
                    out_offset=None,
                    in_=xr_d[:, :],
                    in_offset=IndirectOffsetOnAxis(ap=toki[:, :], axis=0),
                )
                xtg = xtgpool.tile([128, KH * C], BF16, tag="xtg", name="xtg")
                xtg3 = xtg.rearrange("p (k c) -> p k c", k=KH)
                extg.append(xtg)
                for st in range(ST):
                    # [128 slot, H] -> [H, 128 slot] laid as [128, KH, 128]
                    nc.sync.dma_start_transpose(
                        out=xtg3[:, :, st * 128 : (st + 1) * 128],
                        in_=xg[:, st * H : (st + 1) * H],
                    )

            # ---- per-expert FFN + scatter ----
            for e in range(E):
                xtg = extg[e]
                hm_tiles = []
                for ft in range(FT):
                    w1t = w1pool.tile([128, H], BF16, name="w1t")
                    nc.sync.dma_start(out=w1t, in_=w1_d[e, ft])
                    h_ps = psA.tile([128, C], F32, tag="psA", name="h_ps")
                    for kh in range(KH):
                        nc.tensor.matmul(
                            h_ps,
                            w1t[:, kh * 128 : (kh + 1) * 128],
                            xtg[:, kh * C : (kh + 1) * C],
                            start=(kh == 0),
                            stop=(kh == KH - 1),
                        )
                    hm = hpool.tile([128, C], BF16, tag="hm", name="hm")
                    nc.scalar.activation(
                        hm, h_ps, AF.Gelu, bias=b1_sb[:, e * FT + ft : e * FT + ft + 1]
                    )
                    hm_tiles.append(hm)

                o_ps = {}
                for st in range(ST):
                    for hh in range(HH):
                        ps = psB.tile([128, HHW], F32, tag="psB", name="o_ps")
                        # + b2[e] broadcast to every slot (rank-1, K=1)
                        nc.tensor.matmul(
                            ps,
                            ones_bf,
                            b2_sb[:, e * H + hh * HHW : e * H + (hh + 1) * HHW],
                            start=True,
                            stop=False,
                        )
                        o_ps[(st, hh)] = ps
                for ft in range(FT):
                    w2a = w2pool.tile([128, HHW], BF16, name="w2a")
                    nc.sync.dma_start(out=w2a, in_=w2_d[e, 0, ft])
                    w2b = w2pool.tile([128, HHW], BF16, name="w2b")
                    nc.sync.dma_start(out=w2b, in_=w2_d[e, 1, ft])
                    for st in range(ST):
                        lhs = hm_tiles[ft][:, st * 128 : (st + 1) * 128]
                        nc.tensor.matmul(
                            o_ps[(st, 0)], lhs, w2a, start=False, stop=(ft == FT - 1)
                        )
                        nc.tensor.matmul(
                            o_ps[(st, 1)], lhs, w2b, start=False, stop=(ft == FT - 1)
                        )
                o2 = o2pool.tile([128, ST * H], F32, name="o2")
                for st in range(ST):
                    nc.scalar.activation(
                        o2[:, st * H : st * H + HHW], o_ps[(st, 0)], AF.Copy
                    )
                    nc.scalar.activation(
                        o2[:, st * H + HHW : (st + 1) * H], o_ps[(st, 1)], AF.Copy
                    )
                nc.gpsimd.indirect_dma_start(
                    out=y2_dr[:, :],
                    out_offset=IndirectOffsetOnAxis(ap=evals[e][:, :], axis=0),
                    in_=o2.rearrange("p (s h) -> p s h", s=ST),
                    in_offset=None,
                )

            # ---- combine: out[t] = wtop*y2[t] + wbot*y2[TOK+t] ----
            for tt in range(TT):
                nc.sync.dma_start(
                    out=bufA_sb[:, tt * H : (tt + 1) * H],
                    in_=y2_dr[tt * 128 : (tt + 1) * 128, :],
                )
                nc.sync.dma_start(
                    out=bufB_sb[:, tt * H : (tt + 1) * H],
                    in_=y2_dr[TOK + tt * 128 : TOK + (tt + 1) * 128, :],
                )
                accsl = acc_sb[:, tt * H : (tt + 1) * H]
                nc.vector.tensor_scalar(
                    accsl,
                    bufA_sb[:, tt * H : (tt + 1) * H],
                    wt_sb[:, tt : tt + 1],
                    None,
                    AluOpType.mult,
                )
                nc.vector.scalar_tensor_tensor(
                    accsl,
                    bufB_sb[:, tt * H : (tt + 1) * H],
                    wb_sb[:, tt : tt + 1],
                    accsl,
                    AluOpType.mult,
                    AluOpType.add,
                )

            nc.sync.dma_start(out=out_d[:, :], in_=acc_sb[:, :])

    nc.finalize()
    return nc


def _get_module():
    if "nc" not in _CACHE:
        _CACHE["nc"] = _build_routed() if MODE == "routed" else _build_dense()
    return _CACHE["nc"]


def _prep_shared(gate_w, gate_b, w1, b1, w2, b2):
    """Host-side weight rearrangement into DMA-friendly layouts."""
    gw = np.ascontiguousarray(
        gate_w.reshape(KH, 128, E).transpose(1, 0, 2).reshape(128, KH * E),
        dtype=np.float32,
    )
    gb = np.ascontiguousarray(gate_b.reshape(1, E), dtype=np.float32)
    # w1r[e, ft, p, kh*128+c] = w1[e, kh*128+p, ft*128+c]
    w1r = np.ascontiguousarray(
        w1.reshape(E, KH, 128, FT, 128).transpose(0, 3, 2, 1, 4).reshape(E, FT, 128, H),
        dtype=ml_dtypes.bfloat16,
    )
    # w2r[e, hh, ft, p, c] = w2[e, ft*128+p, hh*HHW+c]
    w2r = np.ascontiguousarray(
        w2.reshape(E, FT, 128, HH, HHW).transpose(0, 3, 1, 2, 4),
        dtype=ml_dtypes.bfloat16,
    )
    # b1r[p, e*FT+ft] = b1[e, ft*128+p]
    b1r = np.ascontiguousarray(
        b1.reshape(E, FT, 128).transpose(2, 0, 1).reshape(128, E * FT),
        dtype=np.float32,
    )
    b2r = np.ascontiguousarray(b2, dtype=ml_dtypes.bfloat16)
    b2rr = np.ascontiguousarray(b2.reshape(1, E * H), dtype=ml_dtypes.bfloat16)
    return gw, gb, w1r, w2r, b1r, b2r, b2rr


def kernel(x, gate_w, gate_b, w1, b1, w2, b2, **run_kwargs):
    global LAST_RESULTS
    x = np.asarray(x, dtype=np.float32)
    gw, gb, w1r, w2r, b1r, b2r, b2rr = _prep_shared(
        np.asarray(gate_w, np.float32),
        np.asarray(gate_b, np.float32),
        np.asarray(w1, np.float32),
        np.asarray(b1, np.float32),
        np.asarray(w2, np.float32),
        np.asarray(b2, np.float32),
    )
    xf = x.reshape(N, H)
    iota = np.ascontiguousarray(
        (np.arange(128)[:, None] + 128 * np.arange(TT)[None, :]).astype(np.float32)
    )
    trash = np.full((TOK + 1, 1), TRASH, dtype=np.int32)
    in_maps = []
    for c in range(NCORES):
        xt = np.ascontiguousarray(xf[c * TOK : (c + 1) * TOK, :].T)
        if MODE == "routed":
            xr = np.zeros((TOK + 1, H), dtype=ml_dtypes.bfloat16)
            xr[:TOK] = xf[c * TOK : (c + 1) * TOK, :]
            in_maps.append(
                {
                    "xt": xt,
                    "xr": xr,
                    "gw": gw,
                    "gb": gb,
                    "w1r": w1r,
                    "w2r": w2r,
                    "b1r": b1r,
                    "b2rr": b2rr,
                    "iota": iota,
                    "trash": trash,
                }
            )
        else:
            xtb = np.ascontiguousarray(xt, dtype=ml_dtypes.bfloat16)
            in_maps.append(
                {
                    "xt": xt,
                    "xtb": xtb,
                    "gw": gw,
                    "gb": gb,
                    "w1r": w1r,
                    "w2r": w2r,
                    "b1r": b1r,
                    "b2r": b2r,
                }
            )

    nc = _get_module()
    results = run_bass_kernel_spmd(nc, in_maps, core_ids=list(range(NCORES)), **run_kwargs)
    LAST_RESULTS = results

    out = np.empty((N, H), dtype=np.float32)
    for c in range(NCORES):
        oc = results.results[c]["out"]  # [128, TT*H]
        out[c * TOK : (c + 1) * TOK] = (
            oc.reshape(128, TT, H).transpose(1, 0, 2).reshape(TOK, H)
        )
    return out.reshape(B, S, H)
